# revision 2
# baseline (speedup 1.0000x reference)
"""Trainium2 Bass kernel for nn_Controller (batch-1 two-layer LSTM-cell chain
+ choice head), distributed over 8 NeuronCores.

Math notes (from the module semantics): both LSTMCells run with zero initial
state, so the h @ W_hh.T terms are identically zero and the f-gate multiplies
c=0.  Only the i/g/o thirds of each W_ih are ever needed:
    gates = x @ W_ih.T + (b_ih + b_hh)
    h     = sigmoid(o) * tanh(sigmoid(i) * tanh(g))
That cuts required HBM traffic from 256 MiB to 96 MiB before sharding.

Distribution (v3): layer 0 is row-sharded (each core computes its 256-slot
h0 chunk from 768 gate rows); layer 1 is COLUMN-sharded: each core holds
W1[:, its 256 h0 columns] for all 6144 i/g/o rows and computes a partial
gate pre-activation [6144].  One bf16 AllGather sums the partials; every
core then applies the bias + activations and the choice head locally.

v3 changes over v2 (79.7us):
  * A zero-byte-ish dummy AllGather is triggered first thing on the CC
    stream.  The first collective of a NEFF execution pays ~30us of
    stream-setup barrier + SPMD launch-skew absorption; the dummy pays it
    concurrently with the weight DMA + compute phase, so the real
    AllGather runs at its warm ~5us floor.
  * Weights stream as fp8-e4m3 (x512 host scale, clipped to +-240 for the
    TRN e4m3 format; the 1/512 rides the existing Tanh activation scale).
    Halves HBM traffic: 6.3MB -> 3.15MB.  Stationaries (x, h) stay bf16 --
    the PE upconverts both operands to its internal format, so mixed
    bf16 x fp8 matmuls are legal (only fp32 must pair with fp32).
  * Weight DMAs packed into 8 transfers of [128, 3072B] rows (DMA-dispatch
    on the sync queue costs ~600ns/instruction; 28 small DMAs were
    dispatch-bound).
  * The 12 layer-1 bias matmuls (ones x b1 into psum) are gone: B1 is now
    added once, post-AllGather, as a single fp32 [128,48] vector add.

GEMVs run x-stationary on the PE (stationary = one 128-row x column,
moving = weight slabs).  All sigmoids are computed via
sigmoid(x) = (1 + tanh(x/2))/2 so the scalar engine's activation table is
loaded exactly once (Tanh); the residual *2 factors are folded into the
host-side scaling of W1 and W_choice.
"""

import os
import sys

import numpy as np
import ml_dtypes

for _p in ("/opt/trn_rl_repo", os.path.expanduser("~/.axon_site/_ro/trn_rl_repo")):
    if os.path.isdir(_p) and _p not in sys.path:
        sys.path.insert(0, _p)

import concourse.bass as bass
import concourse.bacc as bacc
import concourse.mybir as mybir
import concourse.tile as tile
from concourse.bass_utils import run_bass_kernel_spmd

H = 2048
NCORES = 8
C = H // NCORES          # 256: per-core h chunk
NK = H // 128            # 16 k-tiles
G1 = 4 * H - H           # 6144 needed gate rows (i/g/o)
CH = 19                  # choice logits
SW = 512.0               # fp8 weight scale (w*SW fits e4m3 comfortably)
DT = mybir.dt.float32
DTW = mybir.dt.bfloat16  # activation-stream dtype
DT8 = mybir.dt.float8e4  # weight-stream dtype
BF = ml_dtypes.bfloat16
F8 = ml_dtypes.float8_e4m3fn
Act = mybir.ActivationFunctionType


# --------------------------------------------------------------------------
# host-side layout prep
# --------------------------------------------------------------------------

def _q8(a):
    """fp8-e4m3 quantize with the TRN +-240 clamp."""
    return np.clip(a, -240.0, 240.0).astype(F8)


def _r0(k):
    """Layer-0 gate rows handled by core k, in [i | o | g] order along the
    768-wide output axis (i,o first so one Tanh op covers both)."""
    a = np.arange(C)
    return np.concatenate([0 * H + k * C + a, 3 * H + k * C + a, 2 * H + k * C + a])


def _r1():
    """Layer-1 global row order: flat index m = q*48 + t reads back as a
    [128, 48] tile with cols [i(16) | o(16) | g(16)] and h-index q*16+u."""
    m = np.arange(G1)
    q, t = m // 48, m % 48
    base = np.where(t < 16, 0, np.where(t < 32, 3 * H, 2 * H))
    u = np.where(t < 16, t, np.where(t < 32, t - 16, t - 32))
    return base + q * 16 + u


def _host_prep(inputs):
    idx = int(np.asarray(inputs["input_idx"]).reshape(-1)[0])
    emb = np.asarray(inputs["embedding"], np.float32)
    x0T = np.ascontiguousarray(emb[idx].reshape(NK, 128).T.astype(BF))

    W0 = np.asarray(inputs["w_ih_0"], np.float32)
    W1 = np.asarray(inputs["w_ih_1"], np.float32)
    B0 = np.asarray(inputs["b_ih_0"], np.float32) + np.asarray(inputs["b_hh_0"], np.float32)
    B1 = np.asarray(inputs["b_ih_1"], np.float32) + np.asarray(inputs["b_hh_1"], np.float32)
    WC = np.asarray(inputs["w_choice"], np.float32)
    BC = np.asarray(inputs["b_choice"], np.float32)

    r1 = _r1()
    # g-gate rows (cols 32:48 of the [128,48] layout) carry an extra x2 so the
    # whole post-AllReduce tile goes through ONE Tanh with scale=0.5/SW:
    # tanh(i/2), tanh(o/2), tanh(g) in a single activation op
    gmul = np.concatenate([np.ones(32), np.full(16, 2.0)])[None, :].repeat(128, 0).reshape(-1)
    # B1 is applied post-AllGather (psum carries SW*gmul*gate, so scale to match)
    b1h = np.ascontiguousarray((B1[r1] * gmul * SW).reshape(128, 48).astype(np.float32))
    # wch[q, u*19+j] = 0.5*WC[j, q*16+u]
    wch = np.ascontiguousarray(
        0.5 * np.transpose(WC.reshape(CH, 128, 16), (1, 2, 0)).reshape(128, 16 * CH)
    ).astype(BF)
    bch = np.ascontiguousarray(BC.reshape(1, CH))
    onesh = np.ones((1, 1), BF)

    maps = []
    for k in range(NCORES):
        R = _r0(k)
        # w0h[t, p, m] = SW*W0[r0[m], t*128+p]; g-rows (m 512:768) carry x2 so
        # both L0 psum chains go through Tanh with a uniform scale=0.5/SW
        w0f = SW * W0[R].T.copy()
        w0f[:, 512:768] *= 2.0
        # pack 4 k-tiles per DMA tile: [4, 128, 4*768]
        w0h = np.ascontiguousarray(
            _q8(w0f.reshape(4, 4, 128, 3 * C).transpose(0, 2, 1, 3).reshape(4, 128, 4 * 3 * C)))
        b0f = SW * B0[R].copy()
        b0f[512:768] *= 2.0
        b0h = np.ascontiguousarray(b0f.reshape(1, 3 * C).astype(BF))
        # layer-1 weights: Ws[m, c] = 0.5*SW*gmul*W1[r1[m], k*256+c]; the
        # stationary h0T holds 2h, so psum = SW*gmul*(W1 @ h) per column chunk.
        Ws = (0.5 * SW * gmul)[:, None] * W1[r1][:, k * C:(k + 1) * C]  # [6144, 256]
        # consumption blocks b = 2i+j: j = h-col (0/1), i = 512-wide... rather:
        # block (j, i) = Ws[i*1024:(i+1)*1024 cols? no -- keep v2's layout:
        # w1[j, i] was [128, 1024] covering gate cols i*1024..(i+1)*1024 for
        # h-half j.  Pack consumption-ordered blocks b=2i+j, 3 per DMA tile.
        w1jik = np.transpose(Ws.reshape(G1, 2, 128), (1, 2, 0)).reshape(2, 128, 6, 1024)
        blocks = [w1jik[j, :, i, :] for i in range(6) for j in range(2)]  # b = 2i+j
        w1h = np.ascontiguousarray(
            _q8(np.stack(blocks).reshape(4, 3, 128, 1024).transpose(0, 2, 1, 3)
                .reshape(4, 128, 3072)))
        maps.append(dict(x0T=x0T, w0=w0h, b0=b0h, w1=w1h, b1=b1h,
                         wc=wch, bc=bch, ones=onesh))
    return maps


# --------------------------------------------------------------------------
# device program (identical on all 8 cores; per-core data differs)
# --------------------------------------------------------------------------

def _build_nc():
    nc = bacc.Bacc("TRN2", target_bir_lowering=False, debug=False,
                   num_devices=NCORES)

    x0T = nc.dram_tensor("x0T", [128, NK], DTW, kind="ExternalInput")
    w0 = nc.dram_tensor("w0", [4, 128, 4 * 3 * C], DT8, kind="ExternalInput")
    b0 = nc.dram_tensor("b0", [1, 3 * C], DTW, kind="ExternalInput")
    w1 = nc.dram_tensor("w1", [4, 128, 3072], DT8, kind="ExternalInput")
    b1 = nc.dram_tensor("b1", [128, 48], DT, kind="ExternalInput")
    wc = nc.dram_tensor("wc", [128, 16 * CH], DTW, kind="ExternalInput")
    bc = nc.dram_tensor("bc", [1, CH], DT, kind="ExternalInput")
    ones = nc.dram_tensor("ones", [1, 1], DTW, kind="ExternalInput")
    out = nc.dram_tensor("out", [CH], DT, kind="ExternalOutput")
    # Shared address space: the HBM-HBM collective fast path
    cc_out = nc.dram_tensor("cc_out", [NCORES * G1], DTW, kind="Internal",
                            addr_space="Shared")
    # dummy-collective buffers: absorb CC-stream setup + launch skew early
    warm_in = nc.dram_tensor("warm_in", [16], DTW, kind="Internal")
    warm_out = nc.dram_tensor("warm_out", [NCORES * 16], DTW, kind="Internal",
                              addr_space="Shared")

    rg = [list(range(NCORES))]

    with tile.TileContext(nc) as tc:
        with (
            tc.tile_pool(name="weights", bufs=1) as wp,
            tc.tile_pool(name="small", bufs=1) as sp,
            tc.tile_pool(name="act", bufs=1) as ap,
            tc.tile_pool(name="psum", bufs=1, space=bass.MemorySpace.PSUM) as pp,
            tc.tile_pool(name="dram", bufs=1, space=bass.MemorySpace.DRAM) as dp,
        ):
            # ---- dummy collective: fire-and-forget, nothing consumes it.
            # ncfw executes stream ops in order, so by the time the real
            # AllGather triggers, stream setup + rank skew are already paid.
            nc.gpsimd.collective_compute(
                "AllGather", mybir.AluOpType.bypass,
                ins=[warm_in.rearrange("(n) -> n").opt()],
                outs=[warm_out.rearrange("(n) -> n").opt()],
                replica_groups=rg,
            )

            # small loads on gpsimd (SWDGE); weight stream owns the sync queue
            onesb = sp.tile([1, 1], DTW, tag="ones")
            nc.gpsimd.dma_start(onesb[:], ones[:])
            x0sb = sp.tile([128, NK], DTW, tag="x0")
            nc.gpsimd.dma_start(x0sb[:], x0T[:])
            b0sb = sp.tile([1, 3 * C], DTW, tag="b0")
            nc.gpsimd.dma_start(b0sb[:], b0[:])
            b1sb = sp.tile([128, 48], DT, tag="b1")
            nc.gpsimd.dma_start(b1sb[:], b1[:])
            wcsb = sp.tile([128, 16 * CH], DTW, tag="wc")
            nc.gpsimd.dma_start(wcsb[:], wc[:])
            bcsb = sp.tile([1, CH], DT, tag="bc")
            nc.gpsimd.dma_start(bcsb[:], bc[:])

            # load the Tanh activation table once, early
            warm = ap.tile([1, 1], DT, tag="warm")
            nc.scalar.activation(warm[:], onesb[:], Act.Tanh)
            # fp32 copy of the ones tile: identity for the fp32 PE transposes
            ones32 = ap.tile([1, 1], DT, tag="ones32")
            nc.vector.tensor_copy(ones32[:], onesb[:])

            # weight stream (sync HWDGE, 16 DMA engines round-robin):
            # 4 + 4 packed fp8 tiles, 3KB per partition row each
            w0sb = []
            for t in range(4):
                wt = wp.tile([128, 4 * 3 * C], DT8, tag=f"w0_{t}")
                nc.sync.dma_start(wt[:], w0[t])
                w0sb.append(wt)
            w1sb = []
            for t in range(4):
                wt = wp.tile([128, 3072], DT8, tag=f"w1_{t}")
                nc.sync.dma_start(wt[:], w1[t])
                w1sb.append(wt)

            def w0mv(t, a, b):
                """moving slab for k-tile t, gate cols a:b (of 768)."""
                return w0sb[t // 4][:, (t % 4) * 768 + a:(t % 4) * 768 + b]

            def w1mv(j, i, c0):
                """moving slab for h-col j, 1024-block i, col offset c0."""
                b = 2 * i + j
                off = (b % 3) * 1024 + c0
                return w1sb[b // 3][:, off:off + 512]

            # ---- layer 0: x-stationary GEMV, 768 rows = [i|o|g], split as
            # two chains so the psum rotation can go deep ----
            psA = pp.tile([1, 512], DT, tag="g4", name="psA")
            psB = pp.tile([1, 512], DT, tag="g5", name="psB")
            nc.tensor.matmul(psA[:], onesb[:], b0sb[:, 0:512], start=True, stop=False)
            nc.tensor.matmul(psB[:, 0:256], onesb[:], b0sb[:, 512:768], start=True, stop=False)
            for t in range(NK):
                nc.tensor.matmul(psA[:], x0sb[:, t:t + 1], w0mv(t, 0, 512),
                                 start=False, stop=(t == NK - 1))
                nc.tensor.matmul(psB[:, 0:256], x0sb[:, t:t + 1], w0mv(t, 512, 768),
                                 start=False, stop=(t == NK - 1))

            # activations: h' = 2h = tanh(c)*(1+tanh(o/2)), c = tanh(g)*(1+tanh(i/2))
            # psum carries SW*(gate or 2*gate): one Tanh with scale 0.5/SW
            t_all0 = ap.tile([1, 768], DT, tag="t_all0")
            nc.scalar.activation(t_all0[:, 0:512], psA[:], Act.Tanh, scale=0.5 / SW)
            nc.scalar.activation(t_all0[:, 512:768], psB[:, 0:256], Act.Tanh, scale=0.5 / SW)
            t_i0, t_o0, t_g0 = t_all0[:, 0:256], t_all0[:, 256:512], t_all0[:, 512:768]
            tmp0 = ap.tile([1, 256], DT, tag="tmp0")
            nc.vector.tensor_mul(tmp0[:], t_g0, t_i0)
            c2 = ap.tile([1, 256], DT, tag="c2")
            nc.vector.tensor_add(c2[:], tmp0[:], t_g0)
            t_c0 = ap.tile([1, 256], DT, tag="t_c0")
            nc.scalar.activation(t_c0[:], c2[:], Act.Tanh, scale=0.5)
            tmp1 = ap.tile([1, 256], DT, tag="tmp1")
            nc.vector.tensor_mul(tmp1[:], t_c0[:], t_o0)
            h0row = ap.tile([1, 256], DT, tag="h0row")
            nc.vector.tensor_add(h0row[:], tmp1[:], t_c0[:])

            # keep the PE busy through the activation gap so the tensor
            # engine's pstate ramp is not reset before the layer-1 burst
            psH = pp.tile([1, CH], DT, tag="psH")
            for _f in range(6):
                nc.tensor.matmul(psH[:], x0sb[:, 0:1], wcsb[:, 0:CH],
                                 start=True, stop=True)

            # transpose h' [1,256] -> [128,2] for the layer-1 stationary x
            # (fp32 transpose: psum writes must stay 4-byte aligned)
            psT = pp.tile([128, 2], DT, tag="psT")
            nc.tensor.transpose(psT[:, 0:1], h0row[0:1, 0:128], ones32[:])
            nc.tensor.transpose(psT[:, 1:2], h0row[0:1, 128:256], ones32[:])
            h0T = ap.tile([128, 2], DTW, tag="h0T")
            nc.vector.tensor_copy(h0T[:], psT[:])

            # ---- layer 1: column-sharded partial gates, 12 groups of 512.
            # B1 is applied post-AllGather, so each group is just 2
            # accumulating matmuls + a drain copy (split vector/scalar) ----
            g1part = sp.tile([1, G1], DTW, tag="g1part")
            cc_in = dp.tile([G1], DTW, tag="cc_in")
            cc_in_v = cc_in.rearrange("(a n) -> a n", a=1)
            Q = G1 // 4
            for g in range(12):
                ps = pp.tile([1, 512], DT, tag=f"g{g % 6}", name=f"ps_g{g}")
                i, c0 = g // 2, (g % 2) * 512
                nc.tensor.matmul(ps[:], h0T[:, 0:1], w1mv(0, i, c0),
                                 start=True, stop=False)
                nc.tensor.matmul(ps[:], h0T[:, 1:2], w1mv(1, i, c0),
                                 start=False, stop=True)
                dst = g1part[:, g * 512:(g + 1) * 512]
                if g % 2:
                    nc.scalar.activation(dst, ps[:], Act.Copy)
                else:
                    nc.vector.tensor_copy(dst, ps[:])
                if g % 3 == 2:
                    # stage the finished quarter on the idle gpsimd SWDGE
                    # queue -- the sync queue is still draining the w1 weight
                    # backlog, and the trigger lives on gpsimd anyway
                    q = g // 3
                    nc.gpsimd.dma_start(cc_in_v[:, q * Q:(q + 1) * Q],
                                        g1part[:, q * Q:(q + 1) * Q])

            # ---- the single collective: AllGather of the partial gates ----
            # (cheaper as a first CC op than AllReduce; the 8-way sum runs on
            # the vector engine in fp32, which is also more accurate than the
            # CC cores' bf16 tree).  The input was staged in 4 chunks above,
            # pipelined behind the drains.
            nc.gpsimd.collective_compute(
                "AllGather", mybir.AluOpType.bypass,
                ins=[cc_in.opt()], outs=[cc_out.rearrange("(n) -> n").opt()],
                replica_groups=rg,
            )
            # readback rank-outer (96B contiguous runs per rank chunk), split
            # across both HWDGE queues (sync + scalar) to overlap dispatch
            g8 = sp.tile([128, NCORES * 48], DTW, tag="g8")
            cc_v = cc_out.rearrange("(r q t) -> q r t", r=NCORES, t=48)
            nc.sync.dma_start(g8[:, 0:192], cc_v[:, 0:4])
            nc.scalar.dma_start(g8[:, 192:384], cc_v[:, 4:8])

            # ---- post-AllGather: 8-way tree sum + bias, contiguous reads ----
            h4 = ap.tile([128, 4 * 48], DT, tag="h4")
            nc.vector.tensor_add(h4[:], g8[:, 0:192], g8[:, 192:384])
            h2 = ap.tile([128, 2 * 48], DT, tag="h2")
            nc.vector.tensor_add(h2[:], h4[:, 0:96], h4[:, 96:192])
            gsum = ap.tile([128, 48], DT, tag="gsum")
            nc.vector.tensor_add(gsum[:], h2[:, 0:48], h2[:, 48:96])
            gsb = ap.tile([128, 48], DT, tag="gsb")
            nc.vector.tensor_add(gsb[:], gsum[:], b1sb[:])
            t_all = ap.tile([128, 48], DT, tag="t_all")
            nc.scalar.activation(t_all[:], gsb[:], Act.Tanh, scale=0.5 / SW)
            t_io1 = t_all  # cols 0:16 = tanh(i/2), 16:32 = tanh(o/2)
            t_g1 = t_all[:, 32:48]  # g rows were pre-doubled: tanh(2g/2) = tanh(g)
            tmpa = ap.tile([128, 16], DT, tag="tmpa")
            nc.vector.tensor_mul(tmpa[:], t_g1, t_io1[:, 0:16])
            c21 = ap.tile([128, 16], DT, tag="c21")
            nc.vector.tensor_add(c21[:], tmpa[:], t_g1)
            t_c1 = ap.tile([128, 16], DT, tag="t_c1")
            nc.scalar.activation(t_c1[:], c21[:], Act.Tanh, scale=0.5)
            tmpb = ap.tile([128, 16], DT, tag="tmpb")
            nc.vector.tensor_mul(tmpb[:], t_c1[:], t_io1[:, 16:32])
            h1 = ap.tile([128, 16], DTW, tag="h1")
            nc.vector.tensor_add(h1[:], tmpb[:], t_c1[:])

            # ---- choice head: logits = 0.5*WC @ h1' + bc, computed locally ----
            for u in range(16):
                nc.tensor.matmul(psH[:], h1[:, u:u + 1], wcsb[:, u * CH:(u + 1) * CH],
                                 start=(u == 0), stop=(u == 15))
            outsb = ap.tile([1, CH], DT, tag="outsb")
            nc.vector.tensor_add(outsb[:], psH[:], bcsb[:])
            nc.sync.dma_start(out.rearrange("(a n) -> a n", a=1), outsb[:])

    nc.compile()
    return nc


_NC_CACHE = None


def _get_nc():
    global _NC_CACHE
    if _NC_CACHE is None:
        _NC_CACHE = _build_nc()
    return _NC_CACHE


# --------------------------------------------------------------------------
# entry point
# --------------------------------------------------------------------------

def kernel(**inputs) -> np.ndarray:
    task = int(np.asarray(inputs["task"]).reshape(-1)[0]) if not isinstance(
        inputs["task"], int) else int(inputs["task"])
    maps = _host_prep(inputs)
    nc = _get_nc()
    for attempt in range(3):
        res = run_bass_kernel_spmd(nc, maps, list(range(NCORES)))
        outs = [np.asarray(res.results[i]["out"], np.float32).reshape(CH)
                for i in range(NCORES)]
        # post-AllReduce every core holds the same logits (up to reduction
        #-order LSBs); gross disagreement means a bad device state -- retry
        if all(np.allclose(outs[0], o, atol=1e-3) for o in outs[1:]):
            break
    logits = outs[0]
    mask = np.arange(CH) < (1 + task)
    return np.where(mask, logits, np.float32(-1e9)).astype(np.float32)


if __name__ == "__main__":
    import reference  # only for standalone debugging; not used by the grader

    inputs = reference.setup_inputs()
    expected = np.asarray(reference.reference(**inputs))
    actual = kernel(**inputs)
    print("expected:", expected)
    print("actual:  ", actual)
    denom = np.abs(expected).max()
    print("max abs err:", np.abs(actual - expected).max(),
          "rel:", np.abs(actual - expected).max() / denom)


# revision 6
# speedup vs baseline: 1.2406x; 1.2406x over previous
"""Trainium2 Bass kernel for nn_Controller (batch-1 two-layer LSTM-cell chain
+ choice head), distributed over 8 NeuronCores.

Math notes (from the module semantics): both LSTMCells run with zero initial
state, so the h @ W_hh.T terms are identically zero and the f-gate multiplies
c=0.  Only the i/g/o thirds of each W_ih are ever needed:
    gates = x @ W_ih.T + (b_ih + b_hh)
    h     = sigmoid(o) * tanh(sigmoid(i) * tanh(g))

Distribution (v5): layer 0 is row-sharded (each core computes its 256-slot
h0 chunk from 768 gate rows); layer 1 is COLUMN-sharded: each core holds
W1[:, its 256 h0 columns] for all 6144 i/g/o rows and computes a partial
gate pre-activation [6144].  One bf16 AllGather shares the partials; every
core then runs the 8-way tree sum in fp32 on its own vector engine, applies
bias + activations and the choice head locally, so the program has exactly
ONE collective.

Known floor (measured): the first collective of a NEFF execution pays a
fixed ~65us ncfw pipeline (stream boot ~21us + entry barrier ~32us +
dispatch ~11us) REGARDLESS of trigger time, so the collective — not the
compute — bounds this kernel.  A remote_dma_broadcast peer exchange would
bypass it (~40us total) but deadlocks the Tile single-core scheduling sim
(wait on a remotely-bumped semaphore is unschedulable), so ncfw it is.

v5 improvements over the 79.7us v2 baseline:
  * Weights stream as fp8-e4m3 (x512 host scale, clipped to +-240 for the
    TRN e4m3 format; the 1/512 rides the Tanh activation scale).  HBM
    traffic halves to 3.15MB/core.  Stationaries (x, h) stay bf16 -- the
    PE upconverts both operands, so mixed bf16 x fp8 matmuls are legal.
  * Weight DMAs packed into 8 transfers of [128, 3072B] rows (DMA dispatch
    costs ~600ns/instruction on the issuing queue; 28 small DMAs were
    dispatch-bound).
  * The 12 layer-1 bias matmuls are gone: B1 is added once, post-AllGather,
    as a single fp32 [128,48] vector add.
  * x0/b0 load on the scalar HWDGE queue (faster than gpsimd SWDGE),
    unblocking the first matmuls sooner.

GEMVs run x-stationary on the PE (stationary = one 128-row x column,
moving = weight slabs).  All sigmoids are computed via
sigmoid(x) = (1 + tanh(x/2))/2 so the scalar engine's activation table is
loaded exactly once (Tanh).
"""

import os
import sys

import numpy as np
import ml_dtypes

for _p in ("/opt/trn_rl_repo", os.path.expanduser("~/.axon_site/_ro/trn_rl_repo")):
    if os.path.isdir(_p) and _p not in sys.path:
        sys.path.insert(0, _p)

import concourse.bass as bass
import concourse.bacc as bacc
import concourse.mybir as mybir
import concourse.tile as tile
from concourse.bass_utils import run_bass_kernel_spmd

H = 2048
NCORES = 8
C = H // NCORES          # 256: per-core h chunk
NK = H // 128            # 16 k-tiles
G1 = 4 * H - H           # 6144 needed gate rows (i/g/o)
CH = 19                  # choice logits
SW = 512.0               # fp8 weight scale (w*SW fits e4m3 comfortably)
DT = mybir.dt.float32
DTW = mybir.dt.bfloat16  # activation-stream dtype
DT8 = mybir.dt.float8e4  # weight-stream dtype
BF = ml_dtypes.bfloat16
F8 = ml_dtypes.float8_e4m3fn
Act = mybir.ActivationFunctionType


# --------------------------------------------------------------------------
# host-side layout prep
# --------------------------------------------------------------------------

def _q8(a):
    """fp8-e4m3 quantize with the TRN +-240 clamp."""
    return np.clip(a, -240.0, 240.0).astype(F8)


def _r0(k):
    """Layer-0 gate rows handled by core k, in [i | o | g] order along the
    768-wide output axis (i,o first so one Tanh op covers both)."""
    a = np.arange(C)
    return np.concatenate([0 * H + k * C + a, 3 * H + k * C + a, 2 * H + k * C + a])


def _r1():
    """Layer-1 global row order: flat index m = q*48 + t reads back as a
    [128, 48] tile with cols [i(16) | o(16) | g(16)] and h-index q*16+u."""
    m = np.arange(G1)
    q, t = m // 48, m % 48
    base = np.where(t < 16, 0, np.where(t < 32, 3 * H, 2 * H))
    u = np.where(t < 16, t, np.where(t < 32, t - 16, t - 32))
    return base + q * 16 + u


def _host_prep(inputs):
    idx = int(np.asarray(inputs["input_idx"]).reshape(-1)[0])
    emb = np.asarray(inputs["embedding"], np.float32)
    x0T = np.ascontiguousarray(emb[idx].reshape(NK, 128).T.astype(BF))

    W0 = np.asarray(inputs["w_ih_0"], np.float32)
    W1 = np.asarray(inputs["w_ih_1"], np.float32)
    B0 = np.asarray(inputs["b_ih_0"], np.float32) + np.asarray(inputs["b_hh_0"], np.float32)
    B1 = np.asarray(inputs["b_ih_1"], np.float32) + np.asarray(inputs["b_hh_1"], np.float32)
    WC = np.asarray(inputs["w_choice"], np.float32)
    BC = np.asarray(inputs["b_choice"], np.float32)

    r1 = _r1()
    # g-gate rows (cols 32:48 of the [128,48] layout) carry an extra x2 so the
    # whole post-exchange tile goes through ONE Tanh with scale=0.5/SW:
    # tanh(i/2), tanh(o/2), tanh(g) in a single activation op
    gmul = np.concatenate([np.ones(32), np.full(16, 2.0)])[None, :].repeat(128, 0).reshape(-1)
    # B1 is applied post-exchange (psum carries SW*gmul*gate, so scale to match)
    b1h = np.ascontiguousarray((B1[r1] * gmul * SW).reshape(128, 48).astype(np.float32))
    # wch[q, u*19+j] = 0.5*WC[j, q*16+u]
    wch = np.ascontiguousarray(
        0.5 * np.transpose(WC.reshape(CH, 128, 16), (1, 2, 0)).reshape(128, 16 * CH)
    ).astype(BF)
    bch = np.ascontiguousarray(BC.reshape(1, CH))
    onesh = np.ones((1, 1), BF)

    maps = []
    for k in range(NCORES):
        R = _r0(k)
        # w0h[t, p, m] = SW*W0[r0[m], t*128+p]; g-rows (m 512:768) carry x2 so
        # both L0 psum chains go through Tanh with a uniform scale=0.5/SW
        w0f = SW * W0[R].T.copy()
        w0f[:, 512:768] *= 2.0
        # pack 4 k-tiles per DMA tile: [4, 128, 4*768]
        w0h = np.ascontiguousarray(
            _q8(w0f.reshape(4, 4, 128, 3 * C).transpose(0, 2, 1, 3).reshape(4, 128, 4 * 3 * C)))
        b0f = SW * B0[R].copy()
        b0f[512:768] *= 2.0
        b0h = np.ascontiguousarray(b0f.reshape(1, 3 * C).astype(BF))
        # layer-1 weights: Ws[m, c] = 0.5*SW*gmul*W1[r1[m], k*256+c]; the
        # stationary h0T holds 2h, so psum = SW*gmul*(W1 @ h) per column chunk.
        Ws = (0.5 * SW * gmul)[:, None] * W1[r1][:, k * C:(k + 1) * C]  # [6144, 256]
        # consumption blocks b = 2i+j (j = h-col 0/1, i = 1024-wide block),
        # packed 3 per DMA tile.
        w1jik = np.transpose(Ws.reshape(G1, 2, 128), (1, 2, 0)).reshape(2, 128, 6, 1024)
        blocks = [w1jik[j, :, i, :] for i in range(6) for j in range(2)]  # b = 2i+j
        w1h = np.ascontiguousarray(
            _q8(np.stack(blocks).reshape(4, 3, 128, 1024).transpose(0, 2, 1, 3)
                .reshape(4, 128, 3072)))
        maps.append(dict(x0T=x0T, w0=w0h, b0=b0h, w1=w1h, b1=b1h,
                         wc=wch, bc=bch, ones=onesh))
    return maps


# --------------------------------------------------------------------------
# device program (identical on all 8 cores; per-core data differs)
# --------------------------------------------------------------------------

def _build_nc():
    nc = bacc.Bacc("TRN2", target_bir_lowering=False, debug=False,
                   num_devices=NCORES)

    x0T = nc.dram_tensor("x0T", [128, NK], DTW, kind="ExternalInput")
    w0 = nc.dram_tensor("w0", [4, 128, 4 * 3 * C], DT8, kind="ExternalInput")
    b0 = nc.dram_tensor("b0", [1, 3 * C], DTW, kind="ExternalInput")
    w1 = nc.dram_tensor("w1", [4, 128, 3072], DT8, kind="ExternalInput")
    b1 = nc.dram_tensor("b1", [128, 48], DT, kind="ExternalInput")
    wc = nc.dram_tensor("wc", [128, 16 * CH], DTW, kind="ExternalInput")
    bc = nc.dram_tensor("bc", [1, CH], DT, kind="ExternalInput")
    ones = nc.dram_tensor("ones", [1, 1], DTW, kind="ExternalInput")
    out = nc.dram_tensor("out", [CH], DT, kind="ExternalOutput")

    # Shared address space: the HBM-HBM collective fast path
    cc_out = nc.dram_tensor("cc_out", [NCORES * G1], DTW, kind="Internal",
                            addr_space="Shared")
    rg = [list(range(NCORES))]

    with tile.TileContext(nc) as tc:
        with (
            tc.tile_pool(name="weights", bufs=1) as wp,
            tc.tile_pool(name="small", bufs=1) as sp,
            tc.tile_pool(name="act", bufs=1) as ap,
            tc.tile_pool(name="psum", bufs=1, space=bass.MemorySpace.PSUM) as pp,
            tc.tile_pool(name="dram", bufs=1, space=bass.MemorySpace.DRAM) as dp,
        ):
            # small loads split across the scalar HWDGE queue (items the first
            # matmuls need) and gpsimd SWDGE (the rest); sync owns the weights
            x0sb = sp.tile([128, NK], DTW, tag="x0")
            nc.scalar.dma_start(x0sb[:], x0T[:])
            b0sb = sp.tile([1, 3 * C], DTW, tag="b0")
            nc.scalar.dma_start(b0sb[:], b0[:])
            onesb = sp.tile([1, 1], DTW, tag="ones")
            nc.gpsimd.dma_start(onesb[:], ones[:])
            b1sb = sp.tile([128, 48], DT, tag="b1")
            nc.gpsimd.dma_start(b1sb[:], b1[:])
            wcsb = sp.tile([128, 16 * CH], DTW, tag="wc")
            nc.gpsimd.dma_start(wcsb[:], wc[:])
            bcsb = sp.tile([1, CH], DT, tag="bc")
            nc.gpsimd.dma_start(bcsb[:], bc[:])

            # load the Tanh activation table once, early
            warm = ap.tile([1, 1], DT, tag="warm")
            nc.scalar.activation(warm[:], onesb[:], Act.Tanh)
            # fp32 copy of the ones tile: identity for the fp32 PE transposes
            ones32 = ap.tile([1, 1], DT, tag="ones32")
            nc.vector.tensor_copy(ones32[:], onesb[:])

            # weight stream (sync HWDGE, 16 DMA engines round-robin):
            # 4 + 4 packed fp8 tiles, 3KB per partition row each
            w0sb = []
            for t in range(4):
                wt = wp.tile([128, 4 * 3 * C], DT8, tag=f"w0_{t}")
                nc.sync.dma_start(wt[:], w0[t])
                w0sb.append(wt)
            w1sb = []
            for t in range(4):
                wt = wp.tile([128, 3072], DT8, tag=f"w1_{t}")
                nc.sync.dma_start(wt[:], w1[t])
                w1sb.append(wt)

            def w0mv(t, a, b):
                """moving slab for k-tile t, gate cols a:b (of 768)."""
                return w0sb[t // 4][:, (t % 4) * 768 + a:(t % 4) * 768 + b]

            def w1mv(j, i, c0):
                """moving slab for h-col j, 1024-block i, col offset c0."""
                b = 2 * i + j
                off = (b % 3) * 1024 + c0
                return w1sb[b // 3][:, off:off + 512]

            # ---- layer 0: x-stationary GEMV, 768 rows = [i|o|g] ----
            psA = pp.tile([1, 512], DT, tag="g4", name="psA")
            psB = pp.tile([1, 512], DT, tag="g5", name="psB")
            nc.tensor.matmul(psA[:], onesb[:], b0sb[:, 0:512], start=True, stop=False)
            nc.tensor.matmul(psB[:, 0:256], onesb[:], b0sb[:, 512:768], start=True, stop=False)
            for t in range(NK):
                nc.tensor.matmul(psA[:], x0sb[:, t:t + 1], w0mv(t, 0, 512),
                                 start=False, stop=(t == NK - 1))
                nc.tensor.matmul(psB[:, 0:256], x0sb[:, t:t + 1], w0mv(t, 512, 768),
                                 start=False, stop=(t == NK - 1))

            # activations: h' = 2h = tanh(c)*(1+tanh(o/2)), c = tanh(g)*(1+tanh(i/2))
            # psum carries SW*(gate or 2*gate): one Tanh with scale 0.5/SW
            t_all0 = ap.tile([1, 768], DT, tag="t_all0")
            nc.scalar.activation(t_all0[:, 0:512], psA[:], Act.Tanh, scale=0.5 / SW)
            nc.scalar.activation(t_all0[:, 512:768], psB[:, 0:256], Act.Tanh, scale=0.5 / SW)
            t_i0, t_o0, t_g0 = t_all0[:, 0:256], t_all0[:, 256:512], t_all0[:, 512:768]
            tmp0 = ap.tile([1, 256], DT, tag="tmp0")
            nc.vector.tensor_mul(tmp0[:], t_g0, t_i0)
            c2 = ap.tile([1, 256], DT, tag="c2")
            nc.vector.tensor_add(c2[:], tmp0[:], t_g0)
            t_c0 = ap.tile([1, 256], DT, tag="t_c0")
            nc.scalar.activation(t_c0[:], c2[:], Act.Tanh, scale=0.5)
            tmp1 = ap.tile([1, 256], DT, tag="tmp1")
            nc.vector.tensor_mul(tmp1[:], t_c0[:], t_o0)
            h0row = ap.tile([1, 256], DT, tag="h0row")
            nc.vector.tensor_add(h0row[:], tmp1[:], t_c0[:])

            # keep the PE busy through the activation gap so the tensor
            # engine's pstate ramp is not reset before the layer-1 burst
            psH = pp.tile([1, CH], DT, tag="psH")
            for _f in range(6):
                nc.tensor.matmul(psH[:], x0sb[:, 0:1], wcsb[:, 0:CH],
                                 start=True, stop=True)

            # transpose h' [1,256] -> [128,2] for the layer-1 stationary x
            # (fp32 transpose: psum writes must stay 4-byte aligned)
            psT = pp.tile([128, 2], DT, tag="psT")
            nc.tensor.transpose(psT[:, 0:1], h0row[0:1, 0:128], ones32[:])
            nc.tensor.transpose(psT[:, 1:2], h0row[0:1, 128:256], ones32[:])
            h0T = ap.tile([128, 2], DTW, tag="h0T")
            nc.vector.tensor_copy(h0T[:], psT[:])

            # ---- layer 1: column-sharded partial gates, 12 groups of 512 ----
            g1part = sp.tile([1, G1], DTW, tag="g1part")
            cc_in = dp.tile([G1], DTW, tag="cc_in")
            cc_in_v = cc_in.rearrange("(a n) -> a n", a=1)
            Q = G1 // 4
            for g in range(12):
                ps = pp.tile([1, 512], DT, tag=f"g{g % 6}", name=f"ps_g{g}")
                i, c0 = g // 2, (g % 2) * 512
                nc.tensor.matmul(ps[:], h0T[:, 0:1], w1mv(0, i, c0),
                                 start=True, stop=False)
                nc.tensor.matmul(ps[:], h0T[:, 1:2], w1mv(1, i, c0),
                                 start=False, stop=True)
                dst = g1part[:, g * 512:(g + 1) * 512]
                if g % 2:
                    nc.scalar.activation(dst, ps[:], Act.Copy)
                else:
                    nc.vector.tensor_copy(dst, ps[:])
                if g % 3 == 2:
                    # stage the finished quarter to HBM (gpsimd SWDGE): the
                    # flat [6144] bounce repartitions the single-row partial
                    # into the [128,48] exchange layout on readback
                    q = g // 3
                    nc.gpsimd.dma_start(cc_in_v[:, q * Q:(q + 1) * Q],
                                        g1part[:, q * Q:(q + 1) * Q])

            # ---- the single collective: AllGather of the partial gates ----
            nc.gpsimd.collective_compute(
                "AllGather", mybir.AluOpType.bypass,
                ins=[cc_in.opt()], outs=[cc_out.rearrange("(n) -> n").opt()],
                replica_groups=rg,
            )
            # readback rank-outer, split across both HWDGE queues
            g8 = sp.tile([128, NCORES * 48], DTW, tag="g8")
            cc_v = cc_out.rearrange("(r q t) -> q r t", r=NCORES, t=48)
            nc.sync.dma_start(g8[:, 0:192], cc_v[:, 0:4])
            nc.scalar.dma_start(g8[:, 192:384], cc_v[:, 4:8])

            # ---- post-AllGather: 8-way tree sum + bias ----
            h4 = ap.tile([128, 4 * 48], DT, tag="h4")
            nc.vector.tensor_add(h4[:], g8[:, 0:192], g8[:, 192:384])
            h2 = ap.tile([128, 2 * 48], DT, tag="h2")
            nc.vector.tensor_add(h2[:], h4[:, 0:96], h4[:, 96:192])
            gsum = ap.tile([128, 48], DT, tag="gsum")
            nc.vector.tensor_add(gsum[:], h2[:, 0:48], h2[:, 48:96])
            gsb = ap.tile([128, 48], DT, tag="gsb")
            nc.vector.tensor_add(gsb[:], gsum[:], b1sb[:])

            t_all = ap.tile([128, 48], DT, tag="t_all")
            nc.scalar.activation(t_all[:], gsb[:], Act.Tanh, scale=0.5 / SW)
            t_io1 = t_all  # cols 0:16 = tanh(i/2), 16:32 = tanh(o/2)
            t_g1 = t_all[:, 32:48]  # g rows were pre-doubled: tanh(2g/2) = tanh(g)
            tmpa = ap.tile([128, 16], DT, tag="tmpa")
            nc.vector.tensor_mul(tmpa[:], t_g1, t_io1[:, 0:16])
            c21 = ap.tile([128, 16], DT, tag="c21")
            nc.vector.tensor_add(c21[:], tmpa[:], t_g1)
            t_c1 = ap.tile([128, 16], DT, tag="t_c1")
            nc.scalar.activation(t_c1[:], c21[:], Act.Tanh, scale=0.5)
            tmpb = ap.tile([128, 16], DT, tag="tmpb")
            nc.vector.tensor_mul(tmpb[:], t_c1[:], t_io1[:, 16:32])
            h1 = ap.tile([128, 16], DTW, tag="h1")
            nc.vector.tensor_add(h1[:], tmpb[:], t_c1[:])

            # ---- choice head: logits = 0.5*WC @ h1' + bc, computed locally ----
            for u in range(16):
                nc.tensor.matmul(psH[:], h1[:, u:u + 1], wcsb[:, u * CH:(u + 1) * CH],
                                 start=(u == 0), stop=(u == 15))
            outsb = ap.tile([1, CH], DT, tag="outsb")
            nc.vector.tensor_add(outsb[:], psH[:], bcsb[:])
            nc.sync.dma_start(out.rearrange("(a n) -> a n", a=1), outsb[:])

    nc.compile()
    return nc


_NC_CACHE = None


def _get_nc():
    global _NC_CACHE
    if _NC_CACHE is None:
        _NC_CACHE = _build_nc()
    return _NC_CACHE


# --------------------------------------------------------------------------
# entry point
# --------------------------------------------------------------------------

def kernel(**inputs) -> np.ndarray:
    task = int(np.asarray(inputs["task"]).reshape(-1)[0]) if not isinstance(
        inputs["task"], int) else int(inputs["task"])
    maps = _host_prep(inputs)
    nc = _get_nc()
    for attempt in range(3):
        res = run_bass_kernel_spmd(nc, maps, list(range(NCORES)))
        outs = [np.asarray(res.results[i]["out"], np.float32).reshape(CH)
                for i in range(NCORES)]
        # post-exchange every core holds the same logits (up to reduction
        #-order LSBs); gross disagreement means a bad device state -- retry
        if all(np.allclose(outs[0], o, atol=1e-3) for o in outs[1:]):
            break
    logits = outs[0]
    mask = np.arange(CH) < (1 + task)
    return np.where(mask, logits, np.float32(-1e9)).astype(np.float32)


if __name__ == "__main__":
    import reference  # only for standalone debugging; not used by the grader

    inputs = reference.setup_inputs()
    expected = np.asarray(reference.reference(**inputs))
    actual = kernel(**inputs)
    print("expected:", expected)
    print("actual:  ", actual)
    denom = np.abs(expected).max()
    print("max abs err:", np.abs(actual - expected).max(),
          "rel:", np.abs(actual - expected).max() / denom)
